# revision 3
# baseline (speedup 1.0000x reference)
"""Trainium2 Bass kernel for nn_CompressedInteractionNet_31997506355236.

Reference math (per batch b, channel k, dim d; m == H == 64, D == 16, vk == 16):
    A[bd, kv] = sum_i x0t[i, bd] * Vm[k, i, v]
    B[bd, kv] = sum_j xhrt[j, bd] * Vh[k, v, j]
    out[bd, k] = sum_v A * B

Strategy: 2D sharding, batch x channels = 4 x 2 over 8 cores (32 batches and
32 output channels per core). Inputs ship as fp16 (tolerance is 2e-2; fp16
keeps ~5e-4) in stacked 128-partition layouts:
    xs [128, 512] = [x0t ; xhrt]          (stationary operands, per batch shard)
    wS [128, 512] = [Vm' ; Vh'] / 2       (moving operand)
    wD [128, 512] = [Vm' ; -Vh'] / 2
One K=128 matmul computes S = (A+B)/2 directly (and D = (A-B)/2), and
    sum_v A*B = sum_v S^2 - D^2
so the two-input elementwise product collapses to ACT Square ops that read
PSUM directly, followed by a DVE grouped reduce and one subtract:
    per 128-row chunk c (4 chunks):
      S = xs_c.T @ wS, D = xs_c.T @ wD        (PE, fp16 in / f32 PSUM pair)
      sq = Square(S | D) -> fp16 SBUF         (ACT, one op over both banks)
      red[s|d] = sum_v sq                      (DVE grouped reduce, fp16 out)
      o = red_s - red_d                        (DVE, f32 out)
The last chunk splits the square/reduce into s/d halves so the output-side
chain starts while the D matmul still runs. All DMAs ride the two HWDGE
queues (sync + scalar) with 128-partition tiles; loads are split in halves
ordered so chunk-0's four quarter matmuls never stall on a late half.
Output leaves as [(b,d), k_loc]; the host unshards/transposes.
"""

import numpy as np

import concourse.bass as bass
import concourse.tile as tile
from concourse import bacc, mybir
from concourse.bass_utils import run_bass_kernel_spmd

# Problem constants (hardcoded; kernel must be self-contained).
B, M, D = 128, 64, 16
HK, VK = 64, 16
H = 64
NCORES = 8
SB, SK = 4, 2             # batch shards x channel shards
BL = B // SB              # batches per core = 32
BD = BL * D               # rows per core = 512
KL = HK // SK             # channels per core = 32
KVL = KL * VK             # 512
NCH = BD // 128           # 128-row chunks per core = 4
F32 = mybir.dt.float32
F16 = mybir.dt.float16

_CACHE = {}


def build_bass():
    nc = bacc.Bacc("TRN2", target_bir_lowering=False, debug=False,
                   num_devices=NCORES, enable_partition_id=False,
                   monotonic_sem_count=0)

    xs_d = nc.dram_tensor("xs", [128, BD], F16, kind="ExternalInput")
    ws_d = nc.dram_tensor("ws", [128, KVL], F16, kind="ExternalInput")
    wd_d = nc.dram_tensor("wd", [128, KVL], F16, kind="ExternalInput")
    out = nc.dram_tensor("out", [BD, KL], F32, kind="ExternalOutput")

    with tile.TileContext(nc) as tc:
        with (
            tc.tile_pool(name="w", bufs=1) as w,
            tc.tile_pool(name="work", bufs=3) as work,
            tc.tile_pool(name="pab", bufs=4, space="PSUM") as pab,
        ):
            # Split loads across the two HWDGE queues, ordered so chunk 0's
            # quarter matmuls (S.a, D.a, S.b, D.b) match arrival order.
            HKV = KVL // 2
            ws = w.tile([128, KVL], F16)
            xs = w.tile([128, BD], F16)
            wd = w.tile([128, KVL], F16)
            nc.scalar.dma_start(ws[:, 0:HKV], ws_d.ap()[:, 0:HKV])
            nc.sync.dma_start(xs[:, 0:256], xs_d.ap()[:, 0:256])
            nc.scalar.dma_start(wd[:, 0:HKV], wd_d.ap()[:, 0:HKV])
            nc.sync.dma_start(ws[:, HKV:KVL], ws_d.ap()[:, HKV:KVL])
            nc.scalar.dma_start(wd[:, HKV:KVL], wd_d.ap()[:, HKV:KVL])
            nc.sync.dma_start(xs[:, 256:512], xs_d.ap()[:, 256:512])

            for c in range(NCH):
                last = c == NCH - 1
                lhsT = xs[:, 128 * c:128 * (c + 1)]
                # psum holds [S | D] adjacent so one ACT op can square both.
                psum = pab.tile([128, 2, KVL], F32, tag="sd")
                if c == 0:
                    # quarter matmuls gated on individual load halves
                    nc.tensor.matmul(psum[:, 0, 0:HKV], lhsT, ws[:, 0:HKV],
                                     start=True, stop=True)
                    nc.tensor.matmul(psum[:, 1, 0:HKV], lhsT, wd[:, 0:HKV],
                                     start=True, stop=True)
                    nc.tensor.matmul(psum[:, 0, HKV:KVL], lhsT, ws[:, HKV:KVL],
                                     start=True, stop=True)
                    nc.tensor.matmul(psum[:, 1, HKV:KVL], lhsT, wd[:, HKV:KVL],
                                     start=True, stop=True)
                else:
                    nc.tensor.matmul(psum[:, 0, :], lhsT, ws[:],
                                     start=True, stop=True)
                    nc.tensor.matmul(psum[:, 1, :], lhsT, wd[:],
                                     start=True, stop=True)

                sq = work.tile([128, 2, KL, VK], F16, tag="sq")
                red = work.tile([128, 2, KL], F16, tag="red")
                with nc.allow_low_precision("fp16 partial sums; tol is 2e-2"):
                    if last:
                        # split s/d so the tail chain starts after the S matmul
                        nc.scalar.activation(
                            sq[:, 0].rearrange("p k v -> p (k v)"), psum[:, 0, :],
                            mybir.ActivationFunctionType.Square)
                        nc.vector.tensor_reduce(out=red[:, 0, :], in_=sq[:, 0],
                                                axis=mybir.AxisListType.X,
                                                op=mybir.AluOpType.add)
                        nc.scalar.activation(
                            sq[:, 1].rearrange("p k v -> p (k v)"), psum[:, 1, :],
                            mybir.ActivationFunctionType.Square)
                        nc.vector.tensor_reduce(out=red[:, 1, :], in_=sq[:, 1],
                                                axis=mybir.AxisListType.X,
                                                op=mybir.AluOpType.add)
                    else:
                        nc.scalar.activation(
                            sq.rearrange("p s k v -> p (s k v)"),
                            psum.rearrange("p s n -> p (s n)"),
                            mybir.ActivationFunctionType.Square)
                        nc.vector.tensor_reduce(out=red[:], in_=sq[:],
                                                axis=mybir.AxisListType.X,
                                                op=mybir.AluOpType.add)
                o_sb = work.tile([128, KL], F32, tag="o")
                nc.vector.tensor_tensor(out=o_sb[:], in0=red[:, 0, :],
                                        in1=red[:, 1, :],
                                        op=mybir.AluOpType.subtract)
                nc.sync.dma_start(out.ap()[128 * c:128 * (c + 1), :], o_sb[:])

    nc.compile()
    return nc


def run(x_0, x_h, Vm, Vh, **spmd_kwargs):
    x_0 = np.ascontiguousarray(np.asarray(x_0), dtype=np.float32)
    vm = np.asarray(Vm)[:, 0].astype(np.float32)   # [k, i, v]
    vh = np.asarray(Vh)[:, 0].astype(np.float32)   # [k, v, j]

    # Host-side layout prep (part of sharding): [i|j, (k,v)] halves, scaled
    # by 0.5 so S^2 - D^2 = A*B without a separate scale op on device.
    vmf = vm.transpose(1, 0, 2).reshape(M, HK * VK) * 0.5
    vhf = vh.transpose(2, 0, 1).reshape(H, HK * VK) * 0.5

    if "nc" not in _CACHE:
        _CACHE["nc"] = build_bass()
    nc = _CACHE["nc"]

    in_maps = []
    for core in range(NCORES):
        cb, ck = divmod(core, SK)
        shard = x_0[BL * cb:BL * (cb + 1)]                    # [BL, M, D]
        x0t = shard.transpose(1, 0, 2).reshape(M, BD)         # [i, (b,d)]
        xhrt = shard.reshape(BL, D, H).transpose(2, 0, 1).reshape(H, BD)
        xs = np.concatenate([x0t, xhrt], axis=0).astype(np.float16)
        ks = slice(KVL * ck, KVL * (ck + 1))
        ws = np.concatenate([vmf[:, ks], vhf[:, ks]], axis=0).astype(np.float16)
        wd = np.concatenate([vmf[:, ks], -vhf[:, ks]], axis=0).astype(np.float16)
        in_maps.append({
            "xs": np.ascontiguousarray(xs),
            "ws": np.ascontiguousarray(ws),
            "wd": np.ascontiguousarray(wd),
        })

    res = run_bass_kernel_spmd(nc, in_maps, core_ids=list(range(NCORES)),
                               **spmd_kwargs)
    # Unshard: per-core out is [(b,d), k_loc] -> [BL, D, KL] -> [BL, KL, D]
    full = np.empty((B, HK, D), dtype=np.float32)
    for core in range(NCORES):
        cb, ck = divmod(core, SK)
        o = res.results[core]["out"].reshape(BL, D, KL).transpose(0, 2, 1)
        full[BL * cb:BL * (cb + 1), KL * ck:KL * (ck + 1), :] = o
    return full, res


def kernel(x_0, x_h, Vm, Vh):
    return run(x_0, x_h, Vm, Vh)[0]


if __name__ == "__main__":
    rng = np.random.default_rng(0)
    x_0 = rng.standard_normal((B, M, D)).astype(np.float32)
    x_h = rng.standard_normal((B, H, D)).astype(np.float32)
    Vm = rng.standard_normal((HK, 1, M, VK)).astype(np.float32)
    Vh = rng.standard_normal((HK, 1, VK, H)).astype(np.float32)
    got = kernel(x_0, x_h, Vm, Vh)

    x0r = np.transpose(x_0, (0, 2, 1))
    xhr = x_0.reshape(B, D, H)
    a = np.einsum("bdi,kiv->bkdv", x0r, Vm[:, 0])
    bb = np.einsum("bdj,kvj->bkdv", xhr, Vh[:, 0])
    want = np.einsum("bkdv,bkdv->bkd", a, bb)
    err = np.abs(got - want).max() / np.abs(want).max()
    print("rel err:", err)


# revision 4
# speedup vs baseline: 1.2070x; 1.2070x over previous
"""Trainium2 Bass kernel for nn_CompressedInteractionNet_31997506355236.

Reference math (per batch b, channel k, dim d; m == H == 64, D == 16, vk == 16):
    A[bd, kv] = sum_i x0t[i, bd] * Vm[k, i, v]
    B[bd, kv] = sum_j xhrt[j, bd] * Vh[k, v, j]
    out[bd, k] = sum_v A * B

Strategy: 2D sharding, batch x channels = 4 x 2 over 8 cores (32 batches and
32 output channels per core). Inputs ship as fp16 (tolerance is 2e-2; fp16
keeps ~5e-4) in stacked 128-partition layouts:
    xs [128, 512] = [x0t ; xhrt]     ws [128, 512] = [Vm' ; Vh']
Per 128-row chunk c (4 chunks), work is spread over four engines:
    A = xs[0:64,c].T @ ws[0:64]   -> PSUM   (PE, K=64)
    B = xs[64:,c].T @ ws[64:]     -> PSUM   (PE, K=64)
    b_sb = copy(B) -> fp16 SBUF             (ACT, reads PSUM)
    p = A * b_sb -> fp16                    (DVE; one PSUM operand)
    t = p[...,0:8] + p[...,8:16]            (GPSIMD fold, chunks 0-2)
    o = sum_v t (or p on the last chunk)    (DVE grouped reduce)
A burst of dummy matmuls on an idle scratch tile keeps the PE busy while the
input DMAs are in flight so the HAM clock gate releases (1.2 -> 2.4 GHz)
before the real matmuls issue. All DMAs ride the two HWDGE queues with
128-partition tiles; loads are split in column halves so chunk 0 only gates
on the first 64 KiB of each stream. Output leaves as [(b,d), k_loc]; the
host unshards/transposes.
"""

import numpy as np

import concourse.bass as bass
import concourse.tile as tile
from concourse import bacc, mybir
from concourse.bass_utils import run_bass_kernel_spmd

# Problem constants (hardcoded; kernel must be self-contained).
B, M, D = 128, 64, 16
HK, VK = 64, 16
H = 64
NCORES = 8
SB, SK = 4, 2             # batch shards x channel shards
BL = B // SB              # batches per core = 32
BD = BL * D               # rows per core = 512
KL = HK // SK             # channels per core = 32
KVL = KL * VK             # 512
NCH = BD // 128           # 128-row chunks per core = 4
F32 = mybir.dt.float32
F16 = mybir.dt.float16

_CACHE = {}


def build_bass():
    nc = bacc.Bacc("TRN2", target_bir_lowering=False, debug=False,
                   num_devices=NCORES, enable_partition_id=False,
                   monotonic_sem_count=0)

    xs_d = nc.dram_tensor("xs", [128, BD], F16, kind="ExternalInput")
    ws_d = nc.dram_tensor("ws", [128, KVL], F16, kind="ExternalInput")
    out = nc.dram_tensor("out", [BD, KL], F32, kind="ExternalOutput")

    with tile.TileContext(nc) as tc:
        with (
            tc.tile_pool(name="w", bufs=1) as w,
            tc.tile_pool(name="work", bufs=3) as work,
            tc.tile_pool(name="pab", bufs=3, space="PSUM") as pab,
            tc.tile_pool(name="dpool", bufs=1, space="PSUM") as dpool,
        ):
            HKV = KVL // 2
            ws = w.tile([128, KVL], F16)
            xs = w.tile([128, BD], F16)
            nc.scalar.dma_start(ws[:, 0:HKV], ws_d.ap()[:, 0:HKV])
            nc.sync.dma_start(xs[:, 0:256], xs_d.ap()[:, 0:256])
            nc.scalar.dma_start(ws[:, HKV:KVL], ws_d.ap()[:, HKV:KVL])
            nc.sync.dma_start(xs[:, 256:512], xs_d.ap()[:, 256:512])

            # PE warm-up: dummy matmuls on a zeroed scratch tile while the
            # loads are in flight, so the HAM clock gate opens before the
            # real matmuls start.
            dumw = w.tile([128, KVL], F16)
            nc.vector.memset(dumw[:], 0.0)
            dpsum = dpool.tile([128, KVL], F32, tag="dummy")
            for _ in range(7):
                nc.tensor.matmul(dpsum[:], dumw[:, 0:128], dumw[:],
                                 start=True, stop=True)

            for c in range(NCH):
                last = c == NCH - 1
                lhsT_a = xs[0:64, 128 * c:128 * (c + 1)]
                lhsT_b = xs[64:128, 128 * c:128 * (c + 1)]
                psum_a = pab.tile([128, KVL], F32, tag="a")
                psum_b = pab.tile([128, KVL], F32, tag="b")
                if c == 0:
                    # halves gated on the two ws load halves
                    nc.tensor.matmul(psum_a[:, 0:HKV], lhsT_a,
                                     ws[0:64, 0:HKV], start=True, stop=True)
                    nc.tensor.matmul(psum_b[:, 0:HKV], lhsT_b,
                                     ws[64:128, 0:HKV], start=True, stop=True)
                    nc.tensor.matmul(psum_a[:, HKV:KVL], lhsT_a,
                                     ws[0:64, HKV:KVL], start=True, stop=True)
                    nc.tensor.matmul(psum_b[:, HKV:KVL], lhsT_b,
                                     ws[64:128, HKV:KVL], start=True, stop=True)
                else:
                    nc.tensor.matmul(psum_a[:], lhsT_a, ws[0:64, :],
                                     start=True, stop=True)
                    nc.tensor.matmul(psum_b[:], lhsT_b, ws[64:128, :],
                                     start=True, stop=True)

                b_sb = work.tile([128, KVL], F16, tag="b_sb")
                nc.scalar.copy(b_sb[:], psum_b[:])
                p = work.tile([128, KL, VK], F16, tag="p")
                nc.vector.tensor_mul(out=p.rearrange("p k v -> p (k v)"),
                                     in0=psum_a[:], in1=b_sb[:])
                o_sb = work.tile([128, KL], F32, tag="o")
                if last:
                    # shortest tail: direct DVE reduce over v=16
                    nc.vector.tensor_reduce(out=o_sb[:], in_=p[:],
                                            axis=mybir.AxisListType.X,
                                            op=mybir.AluOpType.add)
                else:
                    # GPSIMD folds v 16->8, DVE reduces the rest
                    t = work.tile([128, KL, VK // 2], F16, tag="t")
                    nc.gpsimd.tensor_tensor(t[:], p[:, :, 0:8], p[:, :, 8:16],
                                            mybir.AluOpType.add)
                    nc.vector.tensor_reduce(out=o_sb[:], in_=t[:],
                                            axis=mybir.AxisListType.X,
                                            op=mybir.AluOpType.add)
                nc.sync.dma_start(out.ap()[128 * c:128 * (c + 1), :], o_sb[:])

    nc.compile()
    return nc


def run(x_0, x_h, Vm, Vh, **spmd_kwargs):
    x_0 = np.ascontiguousarray(np.asarray(x_0), dtype=np.float32)
    vm = np.asarray(Vm)[:, 0].astype(np.float32)   # [k, i, v]
    vh = np.asarray(Vh)[:, 0].astype(np.float32)   # [k, v, j]

    # Host-side layout prep (part of sharding): [i|j, (k,v)] stacked weights.
    vmf = vm.transpose(1, 0, 2).reshape(M, HK * VK)
    vhf = vh.transpose(2, 0, 1).reshape(H, HK * VK)

    if "nc" not in _CACHE:
        _CACHE["nc"] = build_bass()
    nc = _CACHE["nc"]

    in_maps = []
    for core in range(NCORES):
        cb, ck = divmod(core, SK)
        shard = x_0[BL * cb:BL * (cb + 1)]                    # [BL, M, D]
        x0t = shard.transpose(1, 0, 2).reshape(M, BD)         # [i, (b,d)]
        xhrt = shard.reshape(BL, D, H).transpose(2, 0, 1).reshape(H, BD)
        xs = np.concatenate([x0t, xhrt], axis=0).astype(np.float16)
        ks = slice(KVL * ck, KVL * (ck + 1))
        ws = np.concatenate([vmf[:, ks], vhf[:, ks]], axis=0).astype(np.float16)
        in_maps.append({
            "xs": np.ascontiguousarray(xs),
            "ws": np.ascontiguousarray(ws),
        })

    res = run_bass_kernel_spmd(nc, in_maps, core_ids=list(range(NCORES)),
                               **spmd_kwargs)
    # Unshard: per-core out is [(b,d), k_loc] -> [BL, D, KL] -> [BL, KL, D]
    full = np.empty((B, HK, D), dtype=np.float32)
    for core in range(NCORES):
        cb, ck = divmod(core, SK)
        o = res.results[core]["out"].reshape(BL, D, KL).transpose(0, 2, 1)
        full[BL * cb:BL * (cb + 1), KL * ck:KL * (ck + 1), :] = o
    return full, res


def kernel(x_0, x_h, Vm, Vh):
    return run(x_0, x_h, Vm, Vh)[0]


if __name__ == "__main__":
    rng = np.random.default_rng(0)
    x_0 = rng.standard_normal((B, M, D)).astype(np.float32)
    x_h = rng.standard_normal((B, H, D)).astype(np.float32)
    Vm = rng.standard_normal((HK, 1, M, VK)).astype(np.float32)
    Vh = rng.standard_normal((HK, 1, VK, H)).astype(np.float32)
    got = kernel(x_0, x_h, Vm, Vh)

    x0r = np.transpose(x_0, (0, 2, 1))
    xhr = x_0.reshape(B, D, H)
    a = np.einsum("bdi,kiv->bkdv", x0r, Vm[:, 0])
    bb = np.einsum("bdj,kvj->bkdv", xhr, Vh[:, 0])
    want = np.einsum("bkdv,bkdv->bkd", a, bb)
    err = np.abs(got - want).max() / np.abs(want).max()
    print("rel err:", err)


# revision 6
# speedup vs baseline: 1.2540x; 1.0389x over previous
"""Trainium2 Bass kernel for nn_CompressedInteractionNet_31997506355236.

Reference math (per batch b, channel k, dim d; m == H == 64, D == 16, vk == 16):
    A[bd, kv] = sum_i x0t[i, bd] * Vm[k, i, v]
    B[bd, kv] = sum_j xhrt[j, bd] * Vh[k, v, j]
    out[bd, k] = sum_v A * B

Strategy: 2D sharding, batch x channels = 4 x 2 over 8 cores (32 batches and
32 output channels per core). Inputs ship as fp16 (tolerance is 2e-2; fp16
keeps ~5e-4) in stacked 128-partition layouts:
    xs [128, 512] = [x0t ; xhrt]     ws [128, 512] = [Vm' ; Vh']
Per 128-row chunk c (4 chunks), work is spread over four engines:
    A = xs[0:64,c].T @ ws[0:64]   -> PSUM   (PE, K=64)
    B = xs[64:,c].T @ ws[64:]     -> PSUM   (PE, K=64)
    b_sb = copy(B) -> fp16 SBUF             (ACT, reads PSUM)
    p = A * b_sb -> fp16                    (DVE; one PSUM operand)
    t = p[...,0:8] + p[...,8:16]            (GPSIMD fold, chunks 0-2)
    o = sum_v t (or p on the last chunk)    (DVE grouped reduce)
A burst of dummy matmuls on an idle scratch tile keeps the PE busy while the
input DMAs are in flight so the HAM clock gate releases (1.2 -> 2.4 GHz)
before the real matmuls issue. All DMAs ride the two HWDGE queues with
128-partition tiles; loads are split in column halves so chunk 0 only gates
on the first 64 KiB of each stream. Output leaves as [(b,d), k_loc]; the
host unshards/transposes.
"""

import numpy as np

import concourse.bass as bass
import concourse.tile as tile
from concourse import bacc, mybir
from concourse.bass_utils import run_bass_kernel_spmd

# Problem constants (hardcoded; kernel must be self-contained).
B, M, D = 128, 64, 16
HK, VK = 64, 16
H = 64
NCORES = 8
SB, SK = 4, 2             # batch shards x channel shards
BL = B // SB              # batches per core = 32
BD = BL * D               # rows per core = 512
KL = HK // SK             # channels per core = 32
KVL = KL * VK             # 512
NCH = BD // 128           # 128-row chunks per core = 4
F32 = mybir.dt.float32
F16 = mybir.dt.float16

_CACHE = {}


def build_bass():
    nc = bacc.Bacc("TRN2", target_bir_lowering=False, debug=False,
                   num_devices=NCORES, enable_partition_id=False,
                   monotonic_sem_count=0)

    xs_d = nc.dram_tensor("xs", [128, BD], F16, kind="ExternalInput")
    ws_d = nc.dram_tensor("ws", [128, KVL], F16, kind="ExternalInput")
    out = nc.dram_tensor("out", [BD, KL], F32, kind="ExternalOutput")

    with tile.TileContext(nc) as tc:
        with (
            tc.tile_pool(name="w", bufs=1) as w,
            tc.tile_pool(name="work", bufs=3) as work,
            tc.tile_pool(name="pab", bufs=3, space="PSUM") as pab,
            tc.tile_pool(name="dpool", bufs=1, space="PSUM") as dpool,
        ):
            HKV = KVL // 2
            ws = w.tile([128, KVL], F16)
            xs = w.tile([128, BD], F16)
            nc.scalar.dma_start(ws[:, 0:HKV], ws_d.ap()[:, 0:HKV])
            nc.sync.dma_start(xs[:, 0:256], xs_d.ap()[:, 0:256])
            nc.scalar.dma_start(ws[:, HKV:KVL], ws_d.ap()[:, HKV:KVL])
            nc.sync.dma_start(xs[:, 256:512], xs_d.ap()[:, 256:512])

            # PE warm-up: dummy matmuls on a zeroed scratch tile while the
            # loads are in flight, so the HAM clock gate opens before the
            # real matmuls start.
            dumw = w.tile([128, KVL], F16)
            nc.vector.memset(dumw[:], 0.0)
            dpsum = dpool.tile([128, KVL], F32, tag="dummy")
            for _ in range(2):
                nc.tensor.matmul(dpsum[:], dumw[:, 0:128], dumw[:],
                                 start=True, stop=True)

            for c in range(NCH):
                last = c == NCH - 1
                lhsT_a = xs[0:64, 128 * c:128 * (c + 1)]
                lhsT_b = xs[64:128, 128 * c:128 * (c + 1)]
                psum_a = pab.tile([128, KVL], F32, tag="a")
                psum_b = pab.tile([128, KVL], F32, tag="b")
                b_sb = work.tile([128, KVL], F16, tag="b_sb")
                p = work.tile([128, KL, VK], F16, tag="p")
                o_sb = work.tile([128, KL], F32, tag="o")
                if c == 0:
                    # fully half-split pipeline: each kv half is gated only
                    # on its own ws load half, so the ACT/DVE chain starts
                    # right after the first 64 KiB lands.
                    t = work.tile([128, KL, VK // 2], F16, tag="t")
                    for h in range(2):
                        ksl = slice(HKV * h, HKV * (h + 1))
                        kh = slice(KL // 2 * h, KL // 2 * (h + 1))
                        nc.tensor.matmul(psum_a[:, ksl], lhsT_a, ws[0:64, ksl],
                                         start=True, stop=True)
                        nc.tensor.matmul(psum_b[:, ksl], lhsT_b,
                                         ws[64:128, ksl], start=True, stop=True)
                        nc.scalar.copy(b_sb[:, ksl], psum_b[:, ksl])
                        nc.vector.tensor_mul(
                            out=p[:, kh].rearrange("p k v -> p (k v)"),
                            in0=psum_a[:, ksl], in1=b_sb[:, ksl])
                        nc.gpsimd.tensor_tensor(t[:, kh], p[:, kh, 0:8],
                                                p[:, kh, 8:16],
                                                mybir.AluOpType.add)
                        nc.vector.tensor_reduce(out=o_sb[:, kh], in_=t[:, kh],
                                                axis=mybir.AxisListType.X,
                                                op=mybir.AluOpType.add)
                    nc.sync.dma_start(out.ap()[0:128, :], o_sb[:])
                    continue

                nc.tensor.matmul(psum_a[:], lhsT_a, ws[0:64, :],
                                 start=True, stop=True)
                nc.tensor.matmul(psum_b[:], lhsT_b, ws[64:128, :],
                                 start=True, stop=True)
                nc.scalar.copy(b_sb[:], psum_b[:])
                nc.vector.tensor_mul(out=p.rearrange("p k v -> p (k v)"),
                                     in0=psum_a[:], in1=b_sb[:])
                if last:
                    # shortest tail: direct DVE reduce over v=16
                    nc.vector.tensor_reduce(out=o_sb[:], in_=p[:],
                                            axis=mybir.AxisListType.X,
                                            op=mybir.AluOpType.add)
                else:
                    # GPSIMD folds v 16->8, DVE reduces the rest
                    t = work.tile([128, KL, VK // 2], F16, tag="t")
                    nc.gpsimd.tensor_tensor(t[:], p[:, :, 0:8], p[:, :, 8:16],
                                            mybir.AluOpType.add)
                    nc.vector.tensor_reduce(out=o_sb[:], in_=t[:],
                                            axis=mybir.AxisListType.X,
                                            op=mybir.AluOpType.add)
                nc.sync.dma_start(out.ap()[128 * c:128 * (c + 1), :], o_sb[:])

    nc.compile()
    return nc


def run(x_0, x_h, Vm, Vh, **spmd_kwargs):
    x_0 = np.ascontiguousarray(np.asarray(x_0), dtype=np.float32)
    vm = np.asarray(Vm)[:, 0].astype(np.float32)   # [k, i, v]
    vh = np.asarray(Vh)[:, 0].astype(np.float32)   # [k, v, j]

    # Host-side layout prep (part of sharding): [i|j, (k,v)] stacked weights.
    vmf = vm.transpose(1, 0, 2).reshape(M, HK * VK)
    vhf = vh.transpose(2, 0, 1).reshape(H, HK * VK)

    if "nc" not in _CACHE:
        _CACHE["nc"] = build_bass()
    nc = _CACHE["nc"]

    in_maps = []
    for core in range(NCORES):
        cb, ck = divmod(core, SK)
        shard = x_0[BL * cb:BL * (cb + 1)]                    # [BL, M, D]
        x0t = shard.transpose(1, 0, 2).reshape(M, BD)         # [i, (b,d)]
        xhrt = shard.reshape(BL, D, H).transpose(2, 0, 1).reshape(H, BD)
        xs = np.concatenate([x0t, xhrt], axis=0).astype(np.float16)
        ks = slice(KVL * ck, KVL * (ck + 1))
        ws = np.concatenate([vmf[:, ks], vhf[:, ks]], axis=0).astype(np.float16)
        in_maps.append({
            "xs": np.ascontiguousarray(xs),
            "ws": np.ascontiguousarray(ws),
        })

    res = run_bass_kernel_spmd(nc, in_maps, core_ids=list(range(NCORES)),
                               **spmd_kwargs)
    # Unshard: per-core out is [(b,d), k_loc] -> [BL, D, KL] -> [BL, KL, D]
    full = np.empty((B, HK, D), dtype=np.float32)
    for core in range(NCORES):
        cb, ck = divmod(core, SK)
        o = res.results[core]["out"].reshape(BL, D, KL).transpose(0, 2, 1)
        full[BL * cb:BL * (cb + 1), KL * ck:KL * (ck + 1), :] = o
    return full, res


def kernel(x_0, x_h, Vm, Vh):
    return run(x_0, x_h, Vm, Vh)[0]


if __name__ == "__main__":
    rng = np.random.default_rng(0)
    x_0 = rng.standard_normal((B, M, D)).astype(np.float32)
    x_h = rng.standard_normal((B, H, D)).astype(np.float32)
    Vm = rng.standard_normal((HK, 1, M, VK)).astype(np.float32)
    Vh = rng.standard_normal((HK, 1, VK, H)).astype(np.float32)
    got = kernel(x_0, x_h, Vm, Vh)

    x0r = np.transpose(x_0, (0, 2, 1))
    xhr = x_0.reshape(B, D, H)
    a = np.einsum("bdi,kiv->bkdv", x0r, Vm[:, 0])
    bb = np.einsum("bdj,kvj->bkdv", xhr, Vh[:, 0])
    want = np.einsum("bkdv,bkdv->bkd", a, bb)
    err = np.abs(got - want).max() / np.abs(want).max()
    print("rel err:", err)
